# revision 6
# baseline (speedup 1.0000x reference)
"""MoE MLP (top-2 of 8 experts) Trainium2 Bass kernel, expert-parallel across 8 cores.

Strategy (hardcoded for B=4, L=2048, D=1024, E=8, H=4096, top_k=2, 8 cores):
  - One expert per core. Router replicated: each core receives Wr with columns
    rotated so "its" expert is column 0; top-2 selection/gating is
    rotation-invariant.
  - Router logits computed in fp32 on the PE (lhsT = transposed-x tiles supplied
    by the host as a layout transform; rhs = Wr chunks), top-2 via DVE max8,
    renormalized gate via exp/reciprocal (softmax denominator cancels).
  - Compaction: per-tile cross-partition prefix sums via triangular-matrix
    matmuls; global slot = column prefix + exclusive column-base; unselected
    tokens get slot >= 2^20 and are dropped by DMA bounds checks.
  - Token (id, gate) pairs scattered to a compact DRAM table via indirect DMA
    (one [128,1]-offset scatter per 128-token tile); expert MLP runs over
    capacity C=2560 rows in groups of 512 tokens: indirect-gather x rows (bf16),
    DMA-transpose to [d, t] layout, hT = W1.T @ xT (PE, bf16), SiLU (ACT),
    y = hs.T @ W2 (PE, bf16), gate-scale on ACT, indirect-scatter into a
    zero-filled partial output. Host sums the 8 partial outputs.
"""

import numpy as np
import ml_dtypes

import concourse.bass as bass
import concourse.mybir as mybir
import concourse.tile as tile
from concourse import bacc
from concourse.bass_utils import run_bass_kernel_spmd

F32 = mybir.dt.float32
I32 = mybir.dt.int32
BF16 = mybir.dt.bfloat16
AF = mybir.ActivationFunctionType
ALU = mybir.AluOpType
ts = bass.ts

BIG = float(1 << 20)


def build_moe_kernel(T=8192, D=1024, H=4096, E=8, C=2560, G=512, reps=1, phase='full', ng_limit=None):
    NT = T // 128          # token tiles
    DCH = D // 128         # contraction chunks over D
    HCH = H // 128         # chunks over H
    NG = C // G            # capacity groups
    U = G // 128           # token tiles per group
    ND = D // 512          # 512-wide output column slices

    nc = bacc.Bacc("TRN2", target_bir_lowering=False, debug=False, num_devices=8)

    xT_d = nc.dram_tensor("xT", [D, T], F32, kind="ExternalInput").ap()
    xbf_d = nc.dram_tensor("xbf", [T, D], BF16, kind="ExternalInput").ap()
    Wr_d = nc.dram_tensor("Wr", [D, E], F32, kind="ExternalInput").ap()
    W1_d = nc.dram_tensor("W1", [D, H], BF16, kind="ExternalInput").ap()
    W2_d = nc.dram_tensor("W2", [H, D], BF16, kind="ExternalInput").ap()
    tokid_d = nc.dram_tensor("tokid", [128, NT], F32, kind="ExternalInput").ap()
    ones_d = nc.dram_tensor("ones", [128, 1], F32, kind="ExternalInput").ap()
    triu_d = nc.dram_tensor("triu", [128, 128], F32, kind="ExternalInput").ap()
    triunt_d = nc.dram_tensor("triunt", [NT, NT], F32, kind="ExternalInput").ap()

    out_d = nc.dram_tensor("out", [T, D], F32, kind="ExternalOutput").ap()
    meta_c = nc.dram_tensor("meta_c", [C, 2], F32).ap()

    with tile.TileContext(nc) as tc:
        with (
            tc.tile_pool(name="const", bufs=1) as cp_,
            tc.tile_pool(name="small", bufs=2) as sp_,
            tc.tile_pool(name="w2s", bufs=6) as w2p,
            tc.tile_pool(name="psmall", bufs=2, space="PSUM") as psp,
            tc.tile_pool(name="ph", bufs=2, space="PSUM") as php,
            tc.tile_pool(name="py", bufs=1, space="PSUM") as pyp,
        ):
          for rep in range(reps):
            # ---- persistent constants / weights -------------------------------
            ones_sb = cp_.tile([128, 1], F32)
            nc.sync.dma_start(out=ones_sb[:], in_=ones_d[:])
            triu_sb = cp_.tile([128, 128], F32)
            nc.sync.dma_start(out=triu_sb[:], in_=triu_d[:])
            triunt_sb = cp_.tile([NT, NT], F32)
            nc.sync.dma_start(out=triunt_sb[:], in_=triunt_d[:])
            tokid_sb = cp_.tile([128, NT], F32)
            nc.sync.dma_start(out=tokid_sb[:], in_=tokid_d[:])
            Wr_sb = cp_.tile([128, DCH, E], F32)
            nc.sync.dma_start(out=Wr_sb[:], in_=Wr_d.rearrange("(c p) e -> p c e", p=128))
            W1_sb = cp_.tile([128, DCH, H], BF16)
            nc.sync.dma_start(out=W1_sb[:], in_=W1_d.rearrange("(c p) h -> p c h", p=128))

            sel_all = cp_.tile([128, NT], F32)
            meta_all = cp_.tile([128, NT, 2], F32)
            nc.vector.tensor_copy(
                meta_all[:, :, 0:1].rearrange("p a b -> p (a b)"), tokid_sb[:]
            )

            # ---- phase-scoped init + router ----------------------------------
            with (
                tc.tile_pool(name="zero", bufs=1) as zp,
                tc.tile_pool(name="xrt", bufs=3) as xrp,
            ):
                # zero-fill the meta table; the partial output needs no zero-fill:
                # run_bass_kernel_spmd guarantees pre-zeroed ExternalOutput buffers
                # (native path hands np.zeros to run_neff; PJRT path donates
                # zero-initialized buffers as the output allocation).
                mi = zp.tile([128, C // 128, 2], F32)
                nc.vector.memset(mi[:, :, 0:1], BIG)
                nc.vector.memset(mi[:, :, 1:2], 0.0)
                nc.sync.dma_start(
                    out=meta_c.rearrange("(p i) c -> p i c", p=128), in_=mi[:]
                )

                # router: fp32 logits -> top-2 gate for expert 0 (the rotated own expert)
                xT_r = xT_d.rearrange("(c p) t -> p c t", p=128)
                for i2 in range(NT // 2):  # two token tiles per DMA
                    xt = xrp.tile([128, DCH, 256], F32, tag="xt")
                    nc.sync.dma_start(out=xt[:], in_=xT_r[:, :, ts(i2, 256)])
                    for u in range(2):
                        i = 2 * i2 + u
                        lg_ps = psp.tile([128, E], F32, tag="ps")
                        for c in range(DCH):
                            nc.tensor.matmul(
                                lg_ps[:],
                                lhsT=xt[:, c, ts(u, 128)],
                                rhs=Wr_sb[:, c, :],
                                start=(c == 0),
                                stop=(c == DCH - 1),
                            )
                        lg = sp_.tile([128, E], F32, tag="lg")
                        nc.scalar.copy(lg[:], lg_ps[:])
                        m8 = sp_.tile([128, 8], F32, tag="m8")
                        nc.vector.max(m8[:], lg[:])
                        negv1 = sp_.tile([128, 1], F32, tag="negv1")
                        nc.vector.tensor_scalar_mul(negv1[:], m8[:, 0:1], -1.0)
                        nc.vector.tensor_scalar(
                            out=sel_all[:, i : i + 1],
                            in0=lg[:, 0:1],
                            scalar1=m8[:, 1:2],
                            scalar2=None,
                            op0=ALU.is_ge,
                        )
                        e0 = sp_.tile([128, 1], F32, tag="e0")
                        nc.scalar.activation(e0[:], lg[:, 0:1], AF.Exp, bias=negv1[:, 0:1])
                        ed = sp_.tile([128, 1], F32, tag="ed")
                        nc.scalar.activation(ed[:], m8[:, 1:2], AF.Exp, bias=negv1[:, 0:1])
                        den = sp_.tile([128, 1], F32, tag="den")
                        nc.vector.tensor_scalar_add(den[:], ed[:], 1.0)
                        rden = sp_.tile([128, 1], F32, tag="rden")
                        nc.vector.reciprocal(rden[:], den[:])
                        nc.vector.tensor_tensor(
                            out=meta_all[:, i, 1:2],
                            in0=e0[:],
                            in1=rden[:],
                            op=ALU.mult,
                        )

                # ---- compaction: global slot for every selected token --------
                ct_ps = psp.tile([NT, 1], F32, tag="ps")
                nc.tensor.matmul(ct_ps[:], lhsT=sel_all[:], rhs=ones_sb[:], start=True, stop=True)
                ct_sb = cp_.tile([NT, 1], F32)
                nc.scalar.copy(ct_sb[:], ct_ps[:])
                cb_ps = psp.tile([128, NT], F32, tag="ps")
                nc.tensor.matmul(
                    cb_ps[:],
                    lhsT=ct_sb[:].to_broadcast([NT, 128]),
                    rhs=triunt_sb[:],
                    start=True,
                    stop=True,
                )
                cb_sb = cp_.tile([128, NT], F32)
                nc.scalar.copy(cb_sb[:], cb_ps[:])
                cpr_ps = psp.tile([128, NT], F32, tag="ps")
                nc.tensor.matmul(cpr_ps[:], lhsT=triu_sb[:], rhs=sel_all[:], start=True, stop=True)
                slots_sb = cp_.tile([128, NT], F32)
                nc.vector.tensor_tensor(out=slots_sb[:], in0=cpr_ps[:], in1=cb_sb[:], op=ALU.add)
                big_sb = cp_.tile([128, NT], F32)
                nc.vector.tensor_scalar(
                    out=big_sb[:],
                    in0=sel_all[:],
                    scalar1=-BIG,
                    scalar2=BIG - 1.0,
                    op0=ALU.mult,
                    op1=ALU.add,
                )
                nc.vector.tensor_tensor(out=slots_sb[:], in0=slots_sb[:], in1=big_sb[:], op=ALU.add)
                offs_sb = cp_.tile([128, NT], I32)
                nc.vector.tensor_copy(offs_sb[:], slots_sb[:])

                # scatter (tokid, gate) rows into the compact table
                for i in range(0 if phase == 'norscatter' else NT):
                    nc.gpsimd.indirect_dma_start(
                        out=meta_c[:, :],
                        out_offset=bass.IndirectOffsetOnAxis(ap=offs_sb[:, i : i + 1], axis=0),
                        in_=meta_all[:, i, :],
                        in_offset=None,
                        bounds_check=C - 1,
                        oob_is_err=False,
                    )

            # ---- expert MLP over capacity groups ------------------------------
            if phase == 'router':
                continue
            with tc.tile_pool(name="mlp", bufs=1) as mp:
                for g in range(NG if ng_limit is None else ng_limit):
                    meta_sb = mp.tile([128, U, 2], F32, tag="meta", bufs=2)
                    nc.sync.dma_start(
                        out=meta_sb[:],
                        in_=meta_c[g * G : (g + 1) * G, :].rearrange(
                            "(u p) c -> p u c", p=128
                        ),
                    )
                    idx_sb = mp.tile([128, U], I32, tag="idx", bufs=2)
                    nc.vector.tensor_copy(
                        idx_sb[:], meta_sb[:, :, 0:1].rearrange("p a b -> p (a b)")
                    )
                    xg_sb = mp.tile([128, U, D], BF16, tag="xg", bufs=2)
                    nc.vector.memset(xg_sb[:], 0.0)
                    for u in range(U):
                        nc.gpsimd.indirect_dma_start(
                            out=xg_sb[:, u, :],
                            out_offset=None,
                            in_=xbf_d[:, :],
                            in_offset=bass.IndirectOffsetOnAxis(ap=idx_sb[:, u : u + 1], axis=0),
                            bounds_check=T - 1,
                            oob_is_err=False,
                        )
                    xgT_sb = mp.tile([128, DCH, G], BF16, tag="xgT", bufs=1)
                    for u in range(U):
                        for c in range(DCH):
                            nc.sync.dma_start_transpose(
                                out=xgT_sb[:, c, ts(u, 128)], in_=xg_sb[:, u, ts(c, 128)]
                            )
                    # hT = silu(W1.T @ xT): [H, G] in 128-chunks
                    hsT_sb = mp.tile([128, HCH, G], BF16, tag="hsT", bufs=1)
                    for m in range(HCH):
                        ph = php.tile([128, G], F32, tag="ph")
                        for c in range(DCH):
                            nc.tensor.matmul(
                                ph[:],
                                lhsT=W1_sb[:, c, ts(m, 128)],
                                rhs=xgT_sb[:, c, :],
                                start=(c == 0),
                                stop=(c == DCH - 1),
                            )
                        nc.scalar.activation(hsT_sb[:, m, :], ph[:], AF.Silu)
                    # y = hs.T @ W2: [G, D], gate-scaled on drain
                    yw_sb = mp.tile([128, U, D], F32, tag="yw", bufs=2)
                    for n in range(ND):
                        pys = [
                            pyp.tile([128, 512], F32, tag=f"py{u}", name=f"py{u}_{g}_{n}_{rep}")
                            for u in range(U)
                        ]
                        for m2 in range(HCH):
                            w2t = w2p.tile([128, 512], BF16, tag="w2")
                            nc.sync.dma_start(
                                out=w2t[:], in_=W2_d[ts(m2, 128), ts(n, 512)]
                            )
                            for u in range(U):
                                nc.tensor.matmul(
                                    pys[u][:],
                                    lhsT=hsT_sb[:, m2, ts(u, 128)],
                                    rhs=w2t[:],
                                    start=(m2 == 0),
                                    stop=(m2 == HCH - 1),
                                )
                        for u in range(U):
                            nc.scalar.activation(
                                yw_sb[:, u, ts(n, 512)],
                                pys[u][:],
                                AF.Copy,
                                scale=meta_sb[:, u, 1:2],
                            )
                    for u in range(U):
                        nc.gpsimd.indirect_dma_start(
                            out=out_d[:, :],
                            out_offset=bass.IndirectOffsetOnAxis(ap=idx_sb[:, u : u + 1], axis=0),
                            in_=yw_sb[:, u, :],
                            in_offset=None,
                            bounds_check=T - 1,
                            oob_is_err=False,
                        )
    nc.compile()
    return nc


_NC_CACHE = {}


def _get_nc():
    key = "full"
    if key not in _NC_CACHE:
        _NC_CACHE[key] = build_moe_kernel()
    return _NC_CACHE[key]


def kernel(x, Wr, W1, W2, top_k):
    B, L, D = 4, 2048, 1024
    E, H, T, NT, C = 8, 4096, 8192, 64, 2560
    x = np.asarray(x, dtype=np.float32)
    Wr = np.asarray(Wr, dtype=np.float32)
    W1 = np.asarray(W1, dtype=np.float32)
    W2 = np.asarray(W2, dtype=np.float32)
    assert int(top_k) == 2
    assert x.shape == (B, L, D) and Wr.shape == (D, E)

    xf = np.ascontiguousarray(x.reshape(T, D))
    xT = np.ascontiguousarray(xf.T)
    xbf = np.ascontiguousarray(xf.astype(ml_dtypes.bfloat16))
    tokid = (np.arange(128)[:, None] + 128 * np.arange(NT)[None, :]).astype(np.float32)
    ones = np.ones((128, 1), np.float32)
    q = np.arange(128)
    triu = (q[:, None] <= q[None, :]).astype(np.float32)
    qq = np.arange(NT)
    triunt = (qq[:, None] < qq[None, :]).astype(np.float32)

    nc = _get_nc()
    in_maps = []
    for e in range(E):
        in_maps.append(
            {
                "xT": xT,
                "xbf": xbf,
                "Wr": np.ascontiguousarray(np.roll(Wr, -e, axis=1)),
                "W1": np.ascontiguousarray(W1[e].astype(ml_dtypes.bfloat16)),
                "W2": np.ascontiguousarray(W2[e].astype(ml_dtypes.bfloat16)),
                "tokid": tokid,
                "ones": ones,
                "triu": triu,
                "triunt": triunt,
            }
        )
    res = run_bass_kernel_spmd(nc, in_maps, core_ids=list(range(8)))
    global LAST_RESULTS
    LAST_RESULTS = res
    out = np.zeros((T, D), np.float32)
    for e in range(E):
        out += res.results[e]["out"]
    return out.reshape(B, L, D)


LAST_RESULTS = None
